# revision 1
# baseline (speedup 1.0000x reference)
"""BitLinear (absmean ternary quantized linear) on 8 TRN2 NeuronCores.

out[b,t,o] = sum_i x[b,t,i] * (clip(round(W[o,i]/delta), -1, 1) * delta) + bias[o]
delta = mean(|W|) + 1e-8  over the FULL weight.

Sharding: tensor-parallel over OUT rows (11008 / 8 = 1376 rows per core).
x is replicated. delta partial abs-sums are AllGathered across the 8 cores.
Host passes each core its weight shard transposed ([IN, OUT_SH], contiguous)
so the contraction dim lands on SBUF partitions; host concatenates the 8
output shards.

Quantization without round() (not available on any engine):
  q = clip(round(w/d),-1,1) = 1[w >= d/2] - 1[w <= -d/2]      (a.e.)
    = (sign(w - d/2) + sign(w + d/2)) / 2                      (a.e.)
The matmul distributes over the two threshold maps, so each map (exact in
bf16) feeds its own matmul stream with the scale folded into x:
  out = (d*x) @ a.T - (d*x) @ b.T         [DVE is_ge/is_le method]
  out = (d/2*x) @ s1.T + (d/2*x) @ s2.T   [ACT sign method]
k-tiles are split between the two methods to balance ACT vs DVE time.
bias is added by initializing PSUM with a K=1 matmul ones[1,128].T @ bias.
"""

import numpy as np

B, T, IN, OUT = 8, 16, 4096, 11008
M = B * T               # 128 tokens
CORES = 8
OUT_SH = OUT // CORES   # 1376
KT = IN // 128          # 32 k-tiles
N_TOTAL_W = OUT * IN    # 45088768
EPS = 1e-8

RESIDENT = 28           # k-tiles kept SBUF-resident between pass A and B
NA = 13                 # k-tiles quantized on ACT (sign); rest on DVE (is_ge)
ACT_SET = {round(i * KT / NA) for i in range(NA)}
COL_SLICES = [(0, 512), (512, 1024), (1024, OUT_SH)]

_CACHE = {}


def _build():
    from concourse import bass, bacc, tile, mybir

    f32 = mybir.dt.float32
    bf16 = mybir.dt.bfloat16
    AF = mybir.ActivationFunctionType
    ALU = mybir.AluOpType

    nc = bacc.Bacc("TRN2", target_bir_lowering=False, debug=False, num_devices=CORES)

    wt_d = nc.dram_tensor("wt", [IN, OUT_SH], f32, kind="ExternalInput")
    xt_d = nc.dram_tensor("xt", [IN, M], f32, kind="ExternalInput")
    bias_d = nc.dram_tensor("bias", [1, OUT_SH], f32, kind="ExternalInput")
    out_d = nc.dram_tensor("out", [M, OUT_SH], f32, kind="ExternalOutput")

    with tile.TileContext(nc) as tc:
        with (
            tc.tile_pool(name="wres", bufs=RESIDENT) as wres,
            tc.tile_pool(name="wstream", bufs=2) as wstream,
            tc.tile_pool(name="xp", bufs=1) as xp,
            tc.tile_pool(name="bp", bufs=1) as bp,
            tc.tile_pool(name="cons", bufs=1) as cons,
            tc.tile_pool(name="stat", bufs=1) as stat,
            tc.tile_pool(name="maps", bufs=2) as maps,
            tc.tile_pool(name="xs", bufs=3) as xs,
            tc.tile_pool(name="op", bufs=1) as op,
            tc.tile_pool(name="dram", bufs=1, space="DRAM") as dram,
            tc.tile_pool(name="psmall", bufs=1, space="PSUM") as psmall,
            tc.tile_pool(name="pout", bufs=1, space="PSUM") as pout,
        ):
            # ---- constants / small tiles ----
            ones_col = cons.tile([128, 1], f32)
            ones_row = cons.tile([1, 128], f32)
            nc.vector.memset(ones_col[:], 1.0)
            nc.vector.memset(ones_row[:], 1.0)
            warm = cons.tile([128, 1], f32)
            # pre-load the ACT table set containing Sign while DMAs run
            nc.scalar.activation(warm[:], ones_col[:], AF.Sign)

            partials = stat.tile([128, KT], f32)
            sumP = stat.tile([128, 1], f32)
            s_sb = stat.tile([1, 8], f32)
            gath = stat.tile([8, 8], f32)
            d_sb = stat.tile([1, 1], f32)
            delta_bc = stat.tile([128, 1], f32)
            th = stat.tile([128, 1], f32)       # +delta/2
            nth = stat.tile([128, 1], f32)      # -delta/2
            negd = stat.tile([128, 1], f32)     # -delta

            bias_sb = bp.tile([1, OUT_SH], f32)
            nc.sync.dma_start(out=bias_sb[:], in_=bias_d[:])

            xsb = xp.tile([128, KT, M], f32)
            nc.sync.dma_start(
                out=xsb[:], in_=xt_d[:].rearrange("(t p) c -> p t c", p=128)
            )

            psum_out = pout.tile([M, OUT_SH], f32)

            # bias into PSUM: ones[1,128].T @ bias[1,N] broadcasts bias rows
            for c0, c1 in COL_SLICES:
                nc.tensor.matmul(
                    psum_out[:, c0:c1],
                    ones_row[:],
                    bias_sb[:, c0:c1],
                    start=True,
                    stop=False,
                )

            # ---- pass A: stream W in, abs-sum each tile ----
            w_tiles = {}
            for k in range(KT):
                if k < RESIDENT:
                    wk = wres.tile([128, OUT_SH], f32, tag="w")
                    w_tiles[k] = wk
                else:
                    wk = wstream.tile([128, OUT_SH], f32, tag="ws")
                nc.sync.dma_start(out=wk[:], in_=wt_d[128 * k : 128 * (k + 1), :])
                nc.vector.tensor_reduce(
                    partials[:, k : k + 1],
                    wk[:],
                    axis=mybir.AxisListType.X,
                    op=ALU.add,
                    apply_absolute_value=True,
                )

            # ---- delta: local sum -> AllGather -> total -> broadcast ----
            nc.vector.tensor_reduce(
                sumP[:], partials[:], axis=mybir.AxisListType.X, op=ALU.add
            )
            ps1 = psmall.tile([1, 1], f32, tag="ps1")
            nc.tensor.matmul(ps1[:], sumP[:], ones_col[:])  # sum over partitions
            nc.vector.memset(s_sb[:], 0.0)
            nc.vector.tensor_copy(s_sb[0:1, 0:1], ps1[:])

            cc_in = dram.tile([1, 8], f32)
            cc_out = dram.tile([8, 8], f32, addr_space="Shared")
            nc.gpsimd.dma_start(out=cc_in[:], in_=s_sb[:])
            nc.gpsimd.collective_compute(
                "AllGather",
                ALU.bypass,
                replica_groups=[list(range(CORES))],
                ins=[cc_in[:].opt()],
                outs=[cc_out[:].opt()],
            )
            nc.gpsimd.dma_start(out=gath[:], in_=cc_out[:])

            ps2 = psmall.tile([1, 1], f32, tag="ps2")
            nc.tensor.matmul(ps2[:], gath[0:8, 0:1], ones_col[0:8, 0:1])
            nc.vector.tensor_scalar(
                d_sb[:], ps2[:], 1.0 / N_TOTAL_W, EPS, op0=ALU.mult, op1=ALU.add
            )
            psb = psmall.tile([128, 1], f32, tag="psb")
            nc.tensor.matmul(psb[:], ones_row[:], d_sb[:])  # broadcast delta
            nc.vector.tensor_copy(delta_bc[:], psb[:])
            nc.vector.tensor_scalar_mul(th[:], delta_bc[:], 0.5)
            nc.vector.tensor_scalar_mul(nth[:], delta_bc[:], -0.5)
            nc.vector.tensor_scalar_mul(negd[:], delta_bc[:], -1.0)

            # ---- pass B: quantize + matmul ----
            for k in range(KT):
                if k in w_tiles:
                    wk = w_tiles[k]
                else:
                    wk = wstream.tile([128, OUT_SH], f32, tag="ws")
                    nc.sync.dma_start(
                        out=wk[:], in_=wt_d[128 * k : 128 * (k + 1), :]
                    )
                xk = xsb[:, k, :]
                mA = maps.tile([128, OUT_SH], bf16, tag="mA")
                mB = maps.tile([128, OUT_SH], bf16, tag="mB")
                if k in ACT_SET:
                    # sign method on ACT; both streams share x*(delta/2)
                    nc.scalar.activation(mA[:], wk[:], AF.Sign, bias=nth[:])
                    nc.scalar.activation(mB[:], wk[:], AF.Sign, bias=th[:])
                    xa = xs.tile([128, M], bf16, tag="xa")
                    nc.vector.tensor_scalar_mul(xa[:], xk, th[:])
                    xb = xa
                else:
                    # threshold method on DVE; streams use +delta / -delta
                    nc.vector.tensor_scalar(mA[:], wk[:], th[:], None, op0=ALU.is_ge)
                    nc.vector.tensor_scalar(mB[:], wk[:], nth[:], None, op0=ALU.is_le)
                    xa = xs.tile([128, M], bf16, tag="xa")
                    xb = xs.tile([128, M], bf16, tag="xb")
                    nc.vector.tensor_scalar_mul(xa[:], xk, delta_bc[:])
                    nc.vector.tensor_scalar_mul(xb[:], xk, negd[:])
                last = k == KT - 1
                for c0, c1 in COL_SLICES:
                    nc.tensor.matmul(
                        psum_out[:, c0:c1], xa[:], mA[:, c0:c1], start=False, stop=False
                    )
                for c0, c1 in COL_SLICES:
                    nc.tensor.matmul(
                        psum_out[:, c0:c1], xb[:], mB[:, c0:c1], start=False, stop=last
                    )

            out_sb = op.tile([M, OUT_SH], f32)
            nc.scalar.copy(out_sb[:], psum_out[:])
            nc.sync.dma_start(out=out_d[:], in_=out_sb[:])

    nc.compile()
    return nc


def _get_nc():
    if "nc" not in _CACHE:
        _CACHE["nc"] = _build()
    return _CACHE["nc"]


def _run(x, weight, bias, **spmd_kwargs):
    from concourse.bass_utils import run_bass_kernel_spmd

    x = np.ascontiguousarray(np.asarray(x), dtype=np.float32)
    weight = np.ascontiguousarray(np.asarray(weight), dtype=np.float32)
    bias = np.ascontiguousarray(np.asarray(bias), dtype=np.float32)

    xt = np.ascontiguousarray(x.reshape(M, IN).T)  # [IN, M]
    in_maps = []
    for c in range(CORES):
        rows = slice(c * OUT_SH, (c + 1) * OUT_SH)
        in_maps.append(
            {
                "xt": xt,
                "wt": np.ascontiguousarray(weight[rows].T),  # [IN, OUT_SH]
                "bias": bias[rows].reshape(1, OUT_SH),
            }
        )
    nc = _get_nc()
    res = run_bass_kernel_spmd(nc, in_maps, core_ids=list(range(CORES)), **spmd_kwargs)
    out = np.concatenate([res.results[c]["out"] for c in range(CORES)], axis=1)
    return out.reshape(B, T, OUT).astype(np.float32), res


def kernel(x, weight, bias):
    out, _ = _run(x, weight, bias)
    return out
